# revision 1
# baseline (speedup 1.0000x reference)
"""CrossAttention Trainium2 kernel.

Sharding: 8 cores = 4 batches x 2 head-groups (8 heads each).
Per core: q/k/v projections for its 512-dim head slice, per-head
attention (scores^T orientation, ones-column denominator), out
projection against the matching 512-row slice of wo. Host sums the
two head-group partials per batch and adds bo.

Matmuls run in bf16 (fp32 PSUM accumulation); activations and
normalization run in fp32 on PSUM.
"""

import numpy as np
from contextlib import ExitStack

import concourse.bass as bass
from concourse import bacc
import concourse.tile as tile
import concourse.mybir as mybir
from concourse.bass_utils import run_bass_kernel_spmd

F32 = mybir.dt.float32
BF16 = mybir.dt.bfloat16

S = 2048          # sequence length
D = 1024          # d_model
DS = 512          # per-core head-slice width (8 heads x 64)
H = 8             # heads per core
DH = 64           # head dim
KC = D // 128     # 8 contraction chunks of 128 for the qkv projections
QH = 1024         # query-half size (2 halves of 1024)


def build_nc():
    nc = bacc.Bacc("TRN2")

    xT = nc.declare_dram_parameter("xT", [D, S], BF16, isOutput=False)
    yT = nc.declare_dram_parameter("yT", [D, S], BF16, isOutput=False)
    wq = nc.declare_dram_parameter("wq", [D, DS], BF16, isOutput=False)
    wk = nc.declare_dram_parameter("wk", [D, DS], BF16, isOutput=False)
    wv = nc.declare_dram_parameter("wv", [D, DS], BF16, isOutput=False)
    bq = nc.declare_dram_parameter("bq", [DS], F32, isOutput=False)
    bk = nc.declare_dram_parameter("bk", [DS], F32, isOutput=False)
    bv = nc.declare_dram_parameter("bv", [1, DS], BF16, isOutput=False)
    wo = nc.declare_dram_parameter("wo", [DS, D], BF16, isOutput=False)
    out = nc.declare_dram_parameter("out", [S, D], F32, isOutput=True)

    with tile.TileContext(nc) as tc, ExitStack() as ctx:
        # ---- persistent pools -------------------------------------------
        kv_pool = ctx.enter_context(tc.tile_pool(name="kv", bufs=1))
        wq_pool = ctx.enter_context(tc.tile_pool(name="wqp", bufs=1))
        wo_pool = ctx.enter_context(tc.tile_pool(name="wop", bufs=1))
        const_pool = ctx.enter_context(tc.tile_pool(name="const", bufs=1))
        stream_pool = ctx.enter_context(tc.tile_pool(name="stream", bufs=3))

        # kT: [d, s] per d-block (2 heads stacked per tile)
        kT = [kv_pool.tile([128, S], BF16, tag=f"kT{d}", name=f"kT{d}") for d in range(4)]
        # v: s-tiles [128, 8, 65] — per head 64 v-cols + 1 ones-col
        v_sb = [kv_pool.tile([128, H, DH + 1], BF16, tag=f"v{i}", name=f"v{i}") for i in range(16)]

        wq_sb = wq_pool.tile([128, KC, DS], BF16, tag="wq")
        wo_sb = wo_pool.tile([128, 4, D], BF16, tag="wo")

        bq_sb = const_pool.tile([128, 4], F32, tag="bq")
        bk_sb = const_pool.tile([128, 4], F32, tag="bk")
        bv_sb = const_pool.tile([1, DS], BF16, tag="bv")
        ones_k1 = const_pool.tile([1, 128], BF16, tag="ones_k1")
        ones_b = const_pool.tile([1, DH], BF16, tag="ones_b")

        nc.gpsimd.memset(ones_k1[:], 1.0)
        nc.gpsimd.memset(ones_b[:], 1.0)
        for i in range(16):
            nc.gpsimd.memset(v_sb[i][:, :, DH], 1.0)

        # ---- phase KV: k/v projections (streams yT once) ----------------
        with tc.tile_pool(name="wkv", bufs=1) as wkv_pool, \
             tc.tile_pool(name="pkv", bufs=4, space="PSUM") as pkv:
            wk_sb = wkv_pool.tile([128, KC, DS], BF16, tag="wk")
            wv_sb = wkv_pool.tile([128, KC, DS], BF16, tag="wv")
            nc.sync.dma_start(out=wk_sb[:], in_=wk[:].rearrange("(k p) n -> p k n", p=128))
            nc.sync.dma_start(out=wv_sb[:], in_=wv[:].rearrange("(k p) n -> p k n", p=128))
            nc.sync.dma_start(out=bk_sb[:], in_=bk[:].rearrange("(d p) -> p d", p=128))
            nc.sync.dma_start(out=bv_sb[:], in_=bv[:])

            for sb in range(4):
                if sb == 1:
                    nc.sync.dma_start(out=wq_sb[:], in_=wq[:].rearrange("(k p) n -> p k n", p=128))
                    nc.sync.dma_start(out=bq_sb[:], in_=bq[:].rearrange("(d p) -> p d", p=128))
                elif sb == 2:
                    nc.sync.dma_start(out=wo_sb[:], in_=wo[:].rearrange("(k p) n -> p k n", p=128))
                slab = stream_pool.tile([128, KC, 512], BF16, tag="slab", name="slab")
                nc.sync.dma_start(
                    out=slab[:],
                    in_=yT[:, sb * 512:(sb + 1) * 512].rearrange("(k p) n -> p k n", p=128),
                )
                # kT d-blocks: [128 d, 512 s]
                for d in range(4):
                    ps = pkv.tile([128, 512], F32, tag="pkv", name="pkv")
                    for k0 in range(KC):
                        nc.tensor.matmul(
                            out=ps[:],
                            lhsT=wk_sb[:, k0, d * 128:(d + 1) * 128],
                            rhs=slab[:, k0, :],
                            start=(k0 == 0), stop=(k0 == KC - 1),
                        )
                    nc.vector.tensor_scalar_add(
                        out=kT[d][:, sb * 512:(sb + 1) * 512],
                        in0=ps[:], scalar1=bk_sb[:, d:d + 1],
                    )
                # v s-tiles: [128 s, 512 dv] -> interleaved [128, 8, 65]
                for st in range(4):
                    ps = pkv.tile([128, 512], F32, tag="pkv", name="pkv")
                    for k0 in range(KC):
                        nc.tensor.matmul(
                            out=ps[:],
                            lhsT=slab[:, k0, st * 128:(st + 1) * 128],
                            rhs=wv_sb[:, k0, :],
                            start=(k0 == 0), stop=False,
                        )
                    nc.tensor.matmul(
                        out=ps[:], lhsT=ones_k1[:], rhs=bv_sb[:],
                        start=False, stop=True,
                    )
                    vt = v_sb[sb * 4 + st]
                    nc.vector.tensor_copy(
                        out=vt[:, :, 0:DH],
                        in_=ps[:].rearrange("p (h e) -> p h e", h=H),
                    )

        # ---- main loop over query halves --------------------------------
        # E_all: one-hot head-selector [8, 512] (row h = ones on cols 64h..64h+63)
        # used to broadcast per-head reciprocals onto the matching partitions.
        with tc.tile_pool(name="qh", bufs=2) as qh_pool, \
             tc.tile_pool(name="att", bufs=2) as att_pool, \
             tc.tile_pool(name="pp", bufs=4) as p_pool, \
             tc.tile_pool(name="dn", bufs=2) as dn_pool, \
             tc.tile_pool(name="ost", bufs=3) as ost_pool, \
             tc.tile_pool(name="psc", bufs=2, space="PSUM") as psc, \
             tc.tile_pool(name="pav", bufs=1, space="PSUM") as pav, \
             tc.tile_pool(name="psmall", bufs=2, space="PSUM") as psmall:

            def q_proj(qh, qTh):
                q0 = qh * QH
                for sb2 in range(2):
                    slab = stream_pool.tile([128, KC, 512], BF16, tag="slab", name="slab")
                    c0 = q0 + sb2 * 512
                    nc.sync.dma_start(
                        out=slab[:],
                        in_=xT[:, c0:c0 + 512].rearrange("(k p) n -> p k n", p=128),
                    )
                    for d in range(4):
                        ps = psmall.tile([128, 512], F32, tag="psmall", name="psq")
                        for k0 in range(KC):
                            nc.tensor.matmul(
                                out=ps[:],
                                lhsT=wq_sb[:, k0, d * 128:(d + 1) * 128],
                                rhs=slab[:, k0, :],
                                start=(k0 == 0), stop=(k0 == KC - 1),
                            )
                        nc.vector.tensor_scalar_add(
                            out=qTh[d][:, sb2 * 512:(sb2 + 1) * 512],
                            in0=ps[:], scalar1=bq_sb[:, d:d + 1],
                        )

            def attention(qTh, attnT):
                for h in range(H):
                    dblk, poff = h // 2, 64 * (h % 2)
                    kTh = kT[dblk][poff:poff + 64, :]
                    qThh = qTh[dblk][poff:poff + 64, :]
                    av = pav.tile([DH + 1, QH], F32, tag="pav", name="pav")
                    for ki in range(16):
                        sc = psc.tile([128, QH], F32, tag="psc", name="psc")
                        for half in range(2):
                            nc.tensor.matmul(
                                out=sc[:, half * 512:(half + 1) * 512],
                                lhsT=kTh[:, ki * 128:(ki + 1) * 128],
                                rhs=qThh[:, half * 512:(half + 1) * 512],
                                start=True, stop=True,
                            )
                        pt = p_pool.tile([128, QH], BF16, tag="pt", name="pt")
                        nc.scalar.activation(
                            out=pt[:], in_=sc[:],
                            func=mybir.ActivationFunctionType.Exp, scale=0.125,
                        )
                        for half in range(2):
                            nc.tensor.matmul(
                                out=av[:, half * 512:(half + 1) * 512],
                                lhsT=v_sb[ki][:, h, :],
                                rhs=pt[:, half * 512:(half + 1) * 512],
                                start=(ki == 0), stop=(ki == 15),
                            )
                    # stash unnormalized out^T, then normalize via PE bcast
                    nc.vector.tensor_copy(
                        out=attnT[dblk][poff:poff + 64, :], in_=av[0:DH, :])
                    den = dn_pool.tile([1, QH], F32, tag="den", name="den")
                    nc.vector.tensor_copy(out=den[:], in_=av[DH:DH + 1, :])
                    rec32 = dn_pool.tile([1, QH], F32, tag="rec32", name="rec32")
                    nc.vector.reciprocal_approx_fast(out=rec32[:], in_=den[:])
                    rec16 = dn_pool.tile([1, QH], BF16, tag="rec16", name="rec16")
                    nc.vector.tensor_copy(out=rec16[:], in_=rec32[:])
                    for half in range(2):
                        bc = psmall.tile([128, 512], F32, tag="psmall", name="bc")
                        nc.tensor.matmul(
                            out=bc[0:DH, :], lhsT=ones_b[:],
                            rhs=rec16[0:1, half * 512:(half + 1) * 512],
                            start=True, stop=True,
                        )
                        sl = attnT[dblk][poff:poff + 64, half * 512:(half + 1) * 512]
                        nc.vector.tensor_mul(out=sl, in0=sl, in1=bc[0:DH, :])

            def out_proj(qh, attnT):
                q0 = qh * QH
                for qt in range(8):
                    ost = ost_pool.tile([128, D], F32, tag="ost", name="ost")
                    for nb in range(2):
                        ps = psmall.tile([128, 512], F32, tag="psmall", name="pso")
                        for d in range(4):
                            nc.tensor.matmul(
                                out=ps[:],
                                lhsT=attnT[d][:, qt * 128:(qt + 1) * 128],
                                rhs=wo_sb[:, d, nb * 512:(nb + 1) * 512],
                                start=(d == 0), stop=(d == 3),
                            )
                        nc.vector.tensor_copy(
                            out=ost[:, nb * 512:(nb + 1) * 512], in_=ps[:])
                    r0 = q0 + qt * 128
                    nc.sync.dma_start(out=out[r0:r0 + 128, :], in_=ost[:])

            qTh0 = [qh_pool.tile([128, QH], BF16, tag=f"qTh{d}", name=f"qTh{d}") for d in range(4)]
            att0 = [att_pool.tile([128, QH], BF16, tag=f"attnT{d}", name=f"attnT{d}") for d in range(4)]
            q_proj(0, qTh0)
            attention(qTh0, att0)
            # emit half-1 q-projection before half-0 out-projection: the PE
            # chews through it while the DVE finishes half-0 normalization
            qTh1 = [qh_pool.tile([128, QH], BF16, tag=f"qTh{d}", name=f"qTh{d}b") for d in range(4)]
            q_proj(1, qTh1)
            out_proj(0, att0)
            att1 = [att_pool.tile([128, QH], BF16, tag=f"attnT{d}", name=f"attnT{d}b") for d in range(4)]
            attention(qTh1, att1)
            out_proj(1, att1)

    nc.finalize()
    return nc


_NC_CACHE = {}


def make_in_maps(x, y, wq, wk, wv, bq, bk, bv, wo):
    import ml_dtypes
    bf16 = ml_dtypes.bfloat16
    in_maps = []
    for c in range(8):
        b, hg = c // 2, c % 2
        sl = slice(hg * DS, (hg + 1) * DS)
        in_maps.append({
            "xT": np.ascontiguousarray(x[b].T).astype(bf16),
            "yT": np.ascontiguousarray(y[b].T).astype(bf16),
            "wq": np.ascontiguousarray(wq[:, sl]).astype(bf16),
            "wk": np.ascontiguousarray(wk[:, sl]).astype(bf16),
            "wv": np.ascontiguousarray(wv[:, sl]).astype(bf16),
            "bq": np.ascontiguousarray(bq[sl]).astype(np.float32),
            "bk": np.ascontiguousarray(bk[sl]).astype(np.float32),
            "bv": np.ascontiguousarray(bv[sl]).astype(bf16).reshape(1, DS),
            "wo": np.ascontiguousarray(wo[sl, :]).astype(bf16),
        })
    return in_maps


def kernel(**inputs):
    x = np.asarray(inputs["x"], dtype=np.float32)
    y = np.asarray(inputs["y"], dtype=np.float32)
    wq = np.asarray(inputs["wq"], dtype=np.float32)
    wk = np.asarray(inputs["wk"], dtype=np.float32)
    wv = np.asarray(inputs["wv"], dtype=np.float32)
    wo = np.asarray(inputs["wo"], dtype=np.float32)
    bq = np.asarray(inputs["bq"], dtype=np.float32)
    bk = np.asarray(inputs["bk"], dtype=np.float32)
    bv = np.asarray(inputs["bv"], dtype=np.float32)
    bo = np.asarray(inputs["bo"], dtype=np.float32)

    if "nc" not in _NC_CACHE:
        _NC_CACHE["nc"] = build_nc()
    nc = _NC_CACHE["nc"]

    in_maps = make_in_maps(x, y, wq, wk, wv, bq, bk, bv, wo)
    res = run_bass_kernel_spmd(nc, in_maps, list(range(8)))
    outs = [np.asarray(r["out"], dtype=np.float32) for r in res.results]
    full = np.stack([outs[2 * b] + outs[2 * b + 1] for b in range(4)])
    return (full + bo[None, None, :]).astype(np.float32)



# revision 11
# speedup vs baseline: 1.1533x; 1.1533x over previous
"""CrossAttention Trainium2 kernel.

Sharding: 8 cores = 4 batches x 2 head-groups (8 heads each).
Per core: q/k/v projections for its 512-dim head slice, per-head
attention (scores^T orientation, ones-column denominator), out
projection against the matching 512-row slice of wo. Host sums the
two head-group partials per batch and adds bo (+ bv @ wo, folded out
of the kernel: softmax rows sum to 1, so the v-bias passes through
attention unchanged; the k-bias shifts every score for a query by the
same amount and cancels in softmax, so it is dropped entirely).

Matmuls run in bf16 (fp32 PSUM accumulation). Attention processes q
in blocks of 512; heads run in pairs (2d, 2d+1) whose score matmuls
occupy disjoint 64-row halves of the PE array (tile_position row
groups) and execute concurrently. Each pair's two score tiles share
one [128, 2, 512] PSUM tile so a single N=1024 ACTIVATE(exp) covers
both heads.
"""

import numpy as np
from contextlib import ExitStack

import concourse.bass as bass
from concourse import bacc
import concourse.tile as tile
import concourse.mybir as mybir
from concourse.bass_utils import run_bass_kernel_spmd

F32 = mybir.dt.float32
BF16 = mybir.dt.bfloat16

S = 2048          # sequence length
D = 1024          # d_model
DS = 512          # per-core head-slice width (8 heads x 64)
H = 8             # heads per core
DH = 64           # head dim
KC = D // 128     # 8 contraction chunks of 128 for the qkv projections
QB = 512          # attention q-block width
NQB = S // QB     # 4 q-blocks
NKI = S // 128    # 16 key tiles of 128


def build_nc():
    nc = bacc.Bacc("TRN2")

    xT = nc.declare_dram_parameter("xT", [D, S], BF16, isOutput=False)
    yT = nc.declare_dram_parameter("yT", [D, S], BF16, isOutput=False)
    wq = nc.declare_dram_parameter("wq", [D, DS], BF16, isOutput=False)
    wk = nc.declare_dram_parameter("wk", [D, DS], BF16, isOutput=False)
    wv = nc.declare_dram_parameter("wv", [D, DS], BF16, isOutput=False)
    bq = nc.declare_dram_parameter("bq", [DS], F32, isOutput=False)
    wo = nc.declare_dram_parameter("wo", [DS, D], BF16, isOutput=False)
    out = nc.declare_dram_parameter("out", [S, D], F32, isOutput=True)

    with tile.TileContext(nc) as tc, ExitStack() as ctx:
        # ---- persistent SBUF pools --------------------------------------
        kv_pool = ctx.enter_context(tc.tile_pool(name="kv", bufs=1))
        w_pool = ctx.enter_context(tc.tile_pool(name="wp", bufs=1))
        qt_pool = ctx.enter_context(tc.tile_pool(name="qt", bufs=1))
        att_pool = ctx.enter_context(tc.tile_pool(name="att", bufs=1))
        const_pool = ctx.enter_context(tc.tile_pool(name="const", bufs=1))
        y_pool = ctx.enter_context(tc.tile_pool(name="ysl", bufs=4))
        x_pool = ctx.enter_context(tc.tile_pool(name="xsl", bufs=3))
        pt_pool = ctx.enter_context(tc.tile_pool(name="pt", bufs=14))
        dn_pool = ctx.enter_context(tc.tile_pool(name="dn", bufs=4))
        bcs_pool = ctx.enter_context(tc.tile_pool(name="bcs", bufs=4))
        ost_pool = ctx.enter_context(tc.tile_pool(name="ost", bufs=3))
        # ---- PSUM pools: 4 + 2 + 2 = 8 banks ----------------------------
        psc = ctx.enter_context(tc.tile_pool(name="psc", bufs=2, space="PSUM"))
        pav = ctx.enter_context(tc.tile_pool(name="pav", bufs=2, space="PSUM"))
        psmall = ctx.enter_context(tc.tile_pool(name="psmall", bufs=2, space="PSUM"))

        # kT/qT: [d, s] per d-block; d-block d holds heads 2d (parts 0:64)
        # and 2d+1 (parts 64:128)
        kT = [kv_pool.tile([128, S], BF16, tag=f"kT{d}", name=f"kT{d}") for d in range(4)]
        qT = [qt_pool.tile([128, S], BF16, tag=f"qT{d}", name=f"qT{d}") for d in range(4)]
        attnT = [att_pool.tile([128, S], BF16, tag=f"attnT{d}", name=f"attnT{d}") for d in range(4)]
        # v: s-tiles [128, 8, 65] -- per head 64 v-cols + 1 ones-col (denominator)
        v_sb = [kv_pool.tile([128, H, DH + 1], BF16, tag=f"v{i}", name=f"v{i}") for i in range(NKI)]

        wq_sb = w_pool.tile([128, KC, DS], BF16, tag="wq")
        wk_sb = w_pool.tile([128, KC, DS], BF16, tag="wk")
        wv_sb = w_pool.tile([128, KC, DS], BF16, tag="wv")
        wo_sb = w_pool.tile([128, 4, D], BF16, tag="wo")
        bq_sb = const_pool.tile([128, 4], F32, tag="bq")
        ones_b = const_pool.tile([1, DH], BF16, tag="ones_b")

        nc.gpsimd.memset(ones_b[:], 1.0)
        for i in range(NKI):
            nc.gpsimd.memset(v_sb[i][:, :, DH], 1.0)

        # ---- input DMAs (prefetch order matters for the head phase) -----
        nc.sync.dma_start(out=wk_sb[:], in_=wk[:].rearrange("(k p) n -> p k n", p=128))
        y_slab = [y_pool.tile([128, KC, 512], BF16, tag="yslab", name=f"ysl{s}")
                  for s in range(4)]
        x_slab = [x_pool.tile([128, KC, 512], BF16, tag="xslab", name=f"xsl{s}")
                  for s in range(4)]
        nc.sync.dma_start(out=y_slab[0][:],
                          in_=yT[:, 0:512].rearrange("(k p) n -> p k n", p=128))
        nc.sync.dma_start(out=wq_sb[:], in_=wq[:].rearrange("(k p) n -> p k n", p=128))
        nc.sync.dma_start(out=x_slab[0][:],
                          in_=xT[:, 0:512].rearrange("(k p) n -> p k n", p=128))
        nc.sync.dma_start(out=bq_sb[:], in_=bq[:].rearrange("(d p) -> p d", p=128))
        for s in range(1, 4):
            nc.sync.dma_start(out=y_slab[s][:],
                              in_=yT[:, s * 512:(s + 1) * 512].rearrange("(k p) n -> p k n", p=128))
        nc.sync.dma_start(out=wv_sb[:], in_=wv[:].rearrange("(k p) n -> p k n", p=128))
        nc.sync.dma_start(out=wo_sb[:], in_=wo[:].rearrange("(k p) n -> p k n", p=128))
        for s in range(1, 4):
            nc.sync.dma_start(out=x_slab[s][:],
                              in_=xT[:, s * 512:(s + 1) * 512].rearrange("(k p) n -> p k n", p=128))

        # ---- projection helpers (psmall groups: 1 bank each) ------------
        def k_proj(d, sb):
            ps = psmall.tile([128, 512], F32, tag="psmall", name=f"psk{d}_{sb}")
            for k0 in range(KC):
                nc.tensor.matmul(
                    out=ps[:], lhsT=wk_sb[:, k0, d * 128:(d + 1) * 128],
                    rhs=y_slab[sb][:, k0, :],
                    start=(k0 == 0), stop=(k0 == KC - 1),
                )
            nc.vector.tensor_copy(out=kT[d][:, sb * 512:(sb + 1) * 512], in_=ps[:])

        def q_proj(d, sb):
            ps = psmall.tile([128, 512], F32, tag="psmall", name=f"psq{d}_{sb}")
            for k0 in range(KC):
                nc.tensor.matmul(
                    out=ps[:], lhsT=wq_sb[:, k0, d * 128:(d + 1) * 128],
                    rhs=x_slab[sb][:, k0, :],
                    start=(k0 == 0), stop=(k0 == KC - 1),
                )
            nc.vector.tensor_scalar_add(
                out=qT[d][:, sb * 512:(sb + 1) * 512],
                in0=ps[:], scalar1=bq_sb[:, d:d + 1],
            )

        def v_proj(sb, st):
            # v s-tile: [128 s, 512 dv] -> interleaved [128, 8, 64]
            ps = psmall.tile([128, 512], F32, tag="psmall", name=f"psv{sb}_{st}")
            for k0 in range(KC):
                nc.tensor.matmul(
                    out=ps[:], lhsT=y_slab[sb][:, k0, st * 128:(st + 1) * 128],
                    rhs=wv_sb[:, k0, :],
                    start=(k0 == 0), stop=(k0 == KC - 1),
                )
            vt = v_sb[sb * 4 + st]
            nc.vector.tensor_copy(
                out=vt[:, :, 0:DH],
                in_=ps[:].rearrange("p (h e) -> p h e", h=H),
            )

        def out_proj(qb, qt):
            # out rows q0..q0+128: attnT^T @ wo
            q0 = qb * QB + qt * 128
            for nb in range(2):
                ps = psmall.tile([128, 512], F32, tag="psmall", name=f"pso{qb}_{qt}_{nb}")
                for d in range(4):
                    nc.tensor.matmul(
                        out=ps[:],
                        lhsT=attnT[d][:, q0:q0 + 128],
                        rhs=wo_sb[:, d, nb * 512:(nb + 1) * 512],
                        start=(d == 0), stop=(d == 3),
                    )
                ost = ost_pool.tile([128, 512], F32, tag="ost", name=f"ost{qb}_{qt}_{nb}")
                nc.vector.tensor_copy(out=ost[:], in_=ps[:])
                nc.sync.dma_start(out=out[q0:q0 + 128, nb * 512:(nb + 1) * 512], in_=ost[:])

        # ---- one attention round: head pair d, q-block qb ---------------
        def round_ki(d, qb, with_vproj=False):
            qsl = slice(qb * QB, (qb + 1) * QB)
            av0 = pav.tile([DH + 1, QB], F32, tag="pav", name=f"av0_{d}_{qb}")
            av1 = pav.tile([DH + 1, QB], F32, tag="pav", name=f"av1_{d}_{qb}")
            for ki in range(NKI):
                ksl = slice(ki * 128, (ki + 1) * 128)
                sc2 = psc.tile([128, 2, QB], F32, tag="psc", name=f"sc{d}_{qb}_{ki}")
                # paired score matmuls: rows 0-63 (head 2d) and 64-127
                # (head 2d+1) -> concurrent via PE row tiling
                nc.tensor.matmul(
                    out=sc2[:, 0, :], lhsT=kT[d][0:DH, ksl], rhs=qT[d][0:DH, qsl],
                    start=True, stop=True,
                )
                nc.tensor.matmul(
                    out=sc2[:, 1, :], lhsT=kT[d][DH:128, ksl], rhs=qT[d][DH:128, qsl],
                    start=True, stop=True,
                )
                pt2 = pt_pool.tile([128, 2, QB], BF16, tag="pt", name=f"pt{d}_{qb}_{ki}")
                nc.scalar.activation(
                    out=pt2[:], in_=sc2[:],
                    func=mybir.ActivationFunctionType.Exp, scale=0.125,
                )
                if with_vproj:
                    # first q-block round: v projection rides along; av(ki)
                    # needs exactly v tile (ki//4, ki%4)
                    v_proj(ki // 4, ki % 4)
                nc.tensor.matmul(
                    out=av0[:], lhsT=v_sb[ki][:, 2 * d, :], rhs=pt2[:, 0, :],
                    start=(ki == 0), stop=(ki == NKI - 1),
                )
                nc.tensor.matmul(
                    out=av1[:], lhsT=v_sb[ki][:, 2 * d + 1, :], rhs=pt2[:, 1, :],
                    start=(ki == 0), stop=(ki == NKI - 1),
                )
            return av0, av1

        # normalize: rec = 1/den (den = ones-row 64), broadcast over the
        # 64 dh partitions through the PE, multiply on the copy out
        def round_norm(d, qb, avs):
            for j, av in enumerate(avs):
                poff = DH * j
                asl = attnT[d][poff:poff + DH, qb * QB:(qb + 1) * QB]
                nc.vector.tensor_copy(out=asl, in_=av[0:DH, :])
                den = dn_pool.tile([1, QB], F32, tag="den", name=f"den{d}_{qb}_{j}")
                nc.vector.tensor_copy(out=den[:], in_=av[DH:DH + 1, :])
                rec = dn_pool.tile([1, QB], F32, tag="rec", name=f"rec{d}_{qb}_{j}")
                nc.vector.reciprocal_approx_fast(out=rec[:], in_=den[:])
                rec16 = dn_pool.tile([1, QB], BF16, tag="rec16", name=f"rec16{d}_{qb}_{j}")
                nc.vector.tensor_copy(out=rec16[:], in_=rec[:])
                bc = psmall.tile([128, 512], F32, tag="psmall", name=f"bc{d}_{qb}_{j}")
                nc.tensor.matmul(
                    out=bc[0:DH, :], lhsT=ones_b[:], rhs=rec16[:],
                    start=True, stop=True,
                )
                nc.vector.tensor_mul(out=asl, in0=asl, in1=bc[0:DH, :])

        # ---- schedule ----------------------------------------------------
        # Head: minimum work to unblock round (d=0, qb=0): kT[d0] + qT[d0]
        # slab0. Everything else is emitted as filler between rounds, in an
        # order consistent with both dataflow and pool-buffer rotation.
        for sb in range(4):
            k_proj(0, sb)
        q_proj(0, 0)

        avs = {}
        avs[(0, 0)] = round_ki(0, 0, with_vproj=True)
        for d in range(1, 4):
            for sb in range(4):
                k_proj(d, sb)
            q_proj(d, 0)
            round_norm(d - 1, 0, avs.pop((d - 1, 0)))
            avs[(d, 0)] = round_ki(d, 0)

        # out-proj chunks for block qb-1, spread over block qb's rounds
        op_sched = {0: [0, 1], 1: [2, 3], 2: [], 3: []}
        for qb in range(1, NQB):
            q_proj(0, qb)
            round_norm(3, qb - 1, avs.pop((3, qb - 1)))
            avs[(0, qb)] = round_ki(0, qb)
            for d in range(1, 4):
                q_proj(d, qb)
                for qt in op_sched[d - 1]:
                    out_proj(qb - 1, qt)
                round_norm(d - 1, qb, avs.pop((d - 1, qb)))
                avs[(d, qb)] = round_ki(d, qb)

        round_norm(3, NQB - 1, avs.pop((3, NQB - 1)))
        for qt in range(4):
            out_proj(NQB - 1, qt)

    nc.finalize()
    return nc


_NC_CACHE = {}


def make_in_maps(x, y, wq, wk, wv, bq, wo):
    import ml_dtypes
    bf16 = ml_dtypes.bfloat16
    in_maps = []
    for c in range(8):
        b, hg = c // 2, c % 2
        sl = slice(hg * DS, (hg + 1) * DS)
        in_maps.append({
            "xT": np.ascontiguousarray(x[b].T).astype(bf16),
            "yT": np.ascontiguousarray(y[b].T).astype(bf16),
            "wq": np.ascontiguousarray(wq[:, sl]).astype(bf16),
            "wk": np.ascontiguousarray(wk[:, sl]).astype(bf16),
            "wv": np.ascontiguousarray(wv[:, sl]).astype(bf16),
            "bq": np.ascontiguousarray(bq[sl]).astype(np.float32),
            "wo": np.ascontiguousarray(wo[sl, :]).astype(bf16),
        })
    return in_maps


def kernel(**inputs):
    x = np.asarray(inputs["x"], dtype=np.float32)
    y = np.asarray(inputs["y"], dtype=np.float32)
    wq = np.asarray(inputs["wq"], dtype=np.float32)
    wk = np.asarray(inputs["wk"], dtype=np.float32)
    wv = np.asarray(inputs["wv"], dtype=np.float32)
    wo = np.asarray(inputs["wo"], dtype=np.float32)
    bq = np.asarray(inputs["bq"], dtype=np.float32)
    bv = np.asarray(inputs["bv"], dtype=np.float32)
    bo = np.asarray(inputs["bo"], dtype=np.float32)

    if "nc" not in _NC_CACHE:
        _NC_CACHE["nc"] = build_nc()
    nc = _NC_CACHE["nc"]

    in_maps = make_in_maps(x, y, wq, wk, wv, bq, wo)
    res = run_bass_kernel_spmd(nc, in_maps, list(range(8)))
    outs = [np.asarray(r["out"], dtype=np.float32) for r in res.results]
    full = np.stack([outs[2 * b] + outs[2 * b + 1] for b in range(4)])
    # bk cancels in softmax; bv rides through attention into a constant
    # output offset bv @ wo (softmax rows sum to 1)
    bias = bo + bv @ wo
    return (full + bias[None, None, :]).astype(np.float32)
